# revision 19
# baseline (speedup 1.0000x reference)
"""MoE layer (top-2 of 8 experts) on 8 TRN2 NeuronCores.

Strategy:
  Host: gate logits + softmax + top-2 + renormalized weights (the
      routing / sharding decision), build per-expert token index lists.
  Device (grouped-expert parallel): experts are snake-paired by load
      into 4 groups of 2; each group's 2 cores each run HALF of each of
      the group's experts (two segments per core, token-exact chunk
      widths -- no 128-padding, near-perfect load balance). Per chunk:
      hT = W1-slices.T @ xT (H on partitions), relu via activation,
      then the TRANSPOSED second matmul yT = W2-slab.T @ hT (O on
      partitions, tokens on the free dim) so compute scales with the
      exact token count. The current segment's W1 is SBUF-resident;
      the next segment's W1 overwrites it via pool WAR deps during the
      segment's last chunk; W2 slabs stream per chunk.
  Host: scale columns by the gate weight, transpose, scatter-add.
"""

import numpy as np
import ml_dtypes

import concourse.mybir as mybir
import concourse.tile as tile
from concourse import bacc
from concourse.bass_utils import run_bass_kernel_spmd

P = 128
N_CORES = 8
N_GROUPS = 4                    # core groups
EPG = N_CORES // N_GROUPS       # experts per group == cores per group
BF16 = mybir.dt.bfloat16
F32 = mybir.dt.float32
_bf16_np = ml_dtypes.bfloat16

_build_cache = {}


def _chunks_of(T):
    """Near-equal chunks <= 512 wide; >= ~200 so LDWEIGHTS stays hidden."""
    n = -(-T // 512)
    base = T // n
    rem = T - base * n
    out = [base + (1 if i < rem else 0) for i in range(n)]
    assert all(c >= 200 for c in out), out
    return out


def _build_expertN(D, H, O, segs):
    """Multi-expert core: segs = tuple of per-segment chunk-width tuples.

    Inputs: xc [D, sum(all widths)] bf16 column-packed tokens; per
    segment s: w1_s [D, H] bf16; w2_s [O, H] bf16 slab-major
    (w2_s[ot*P + p_h, kt*P + oc] = W2[kt*P + p_h, ot*P + oc]);
    b1_s [P, H/P] f32; b2_s [P, O/P] f32.
    Output: yT [O, TT] f32 = (relu(x W1 + b1) W2 + b2).T, unscaled.
    """
    nc = bacc.Bacc(None, target_bir_lowering=False)
    NS = len(segs)
    TT = sum(sum(sg) for sg in segs)
    xc = nc.dram_tensor("xc", [D, TT], BF16, kind="ExternalInput")
    w1_d = [nc.dram_tensor(f"w1_{s}", [D, H], BF16, kind="ExternalInput")
            for s in range(NS)]
    w2_d = [nc.dram_tensor(f"w2_{s}", [O, H], BF16, kind="ExternalInput")
            for s in range(NS)]
    b1_d = [nc.dram_tensor(f"b1_{s}", [P, H // P], F32, kind="ExternalInput")
            for s in range(NS)]
    b2_d = [nc.dram_tensor(f"b2_{s}", [P, O // P], F32, kind="ExternalInput")
            for s in range(NS)]
    yT = nc.dram_tensor("yT", [O, TT], F32, kind="ExternalOutput")

    DO, HO, OT = D // P, H // P, O // P
    HG = 8
    NHG = HO // HG
    CW = 512
    xc_r = xc.rearrange("(do p) t -> p do t", p=P)
    w2_rs = [w.rearrange("(ot p) c -> p ot c", p=P) for w in w2_d]
    yT_r = yT.rearrange("(ot p) t -> p ot t", p=P)
    w1_rs = [w.rearrange("(do p) h -> p do h", p=P) for w in w1_d]

    w1_chunks = [(h, 1) for h in range(4)] + [(h, 4) for h in range(4, HO, 4)]
    w1_of_hi = {}
    for ci, (h0, nh) in enumerate(w1_chunks):
        for j in range(nh):
            w1_of_hi[h0 + j] = (ci, j)
    NW1 = len(w1_chunks)

    with tile.TileContext(nc) as tc:
        with (
            tc.tile_pool(name="w1pool", bufs=1) as w1p,
            tc.tile_pool(name="w2pool", bufs=3) as w2p,
            tc.tile_pool(name="cpool", bufs=1) as cp,
            tc.tile_pool(name="xpool", bufs=2) as xp,
            tc.tile_pool(name="hpool", bufs=2) as hp,
            tc.tile_pool(name="opool", bufs=4) as op,
            tc.tile_pool(name="hps", bufs=4, space="PSUM") as hps,
            tc.tile_pool(name="yps", bufs=3, space="PSUM") as yps,
        ):
            x0_sb = xp.tile([P, DO, CW], BF16, tag="x", name="x0_sb")
            w0 = segs[0][0]
            # first half of x0 leads the sync ring (ahead of W1), second
            # half leads scalar: both land ~in parallel, W1 follows
            nc.sync.dma_start(x0_sb[:, 0:4, :w0], xc_r[:, 0:4, 0:w0])
            nc.scalar.dma_start(x0_sb[:, 4:8, :w0], xc_r[:, 4:8, 0:w0])
            b1_sb = [cp.tile([P, HO], F32, tag=f"b1{s}", name=f"b1{s}")
                     for s in range(NS)]
            nc.scalar.dma_start(b1_sb[0][:], b1_d[0][:])
            w1t = {0: [w1p.tile([P, DO, nh * P], BF16, tag=f"w1_{k}",
                               name=f"w1g0_{k}")
                       for k, (h0, nh) in enumerate(w1_chunks)]}
            for k, (h0, nh) in enumerate(w1_chunks):
                nc.sync.dma_start(w1t[0][k][:],
                                  w1_rs[0][:, :, h0 * P:(h0 + nh) * P])
            b2_sb = [cp.tile([P, OT], F32, tag=f"b2{s}", name=f"b2{s}")
                     for s in range(NS)]
            for s in range(1, NS):
                nc.scalar.dma_start(b1_sb[s][:], b1_d[s][:])
            for s in range(NS):
                nc.scalar.dma_start(b2_sb[s][:], b2_d[s][:])

            chunks = []
            pos = 0
            for s in range(NS):
                for j, w in enumerate(segs[s]):
                    chunks.append((s, pos, w, j == len(segs[s]) - 1))
                    pos += w

            def emit_w1(snext, k0, k1):
                # gen tiles share addresses; pool WAR deps hold each load
                # until the last reader of the previous generation
                for k in range(k0, k1):
                    h0, nh = w1_chunks[k]
                    t = w1p.tile([P, DO, nh * P], BF16, tag=f"w1_{k}",
                                 name=f"w1g{snext}_{k}")
                    nc.sync.dma_start(t[:],
                                      w1_rs[snext][:, :, h0 * P:(h0 + nh) * P])
                    w1t.setdefault(snext, []).append(t)

            for cidx, (s, c0, w, is_last) in enumerate(chunks):
                if cidx == 0:
                    x_sb = x0_sb[:, :, :w]
                else:
                    x_sb = xp.tile([P, DO, CW], BF16, tag="x",
                                   name="x_sb")[:, :, :w]
                    nc.sync.dma_start(x_sb[:], xc_r[:, :, c0:c0 + w])
                hgs = [hp.tile([P, HG, CW], BF16, tag=f"h{g}",
                               name=f"h{g}")[:, :, :w] for g in range(NHG)]
                acts = []
                # ---- mm1: h[hi] = relu(W1[:, hi].T @ x + b1) ----
                for hi in range(HO):
                    ph = hps.tile([P, CW], F32, tag="ph", name="ph")[:, :w]
                    ci, off = w1_of_hi[hi]
                    for di in range(DO):
                        nc.tensor.matmul(
                            ph[:],
                            w1t[s][ci][:, di, off * P:(off + 1) * P],
                            x_sb[:, di],
                            start=(di == 0),
                            stop=(di == DO - 1),
                        )
                    acts.append(nc.scalar.activation(
                        hgs[hi // HG][:, hi % HG], ph[:],
                        mybir.ActivationFunctionType.Relu,
                        bias=b1_sb[s][:, hi:hi + 1],
                    ))
                if is_last and s + 1 < NS:
                    emit_w1(s + 1, 0, NW1 // 2)
                # ---- mm2 (transposed): yT[ot] = W2-slab[ot].T @ h ----
                for ot in range(OT):
                    slab = w2p.tile([P, HO, P], BF16, tag="w2s", name="w2s")
                    dma = nc.scalar.dma_start(
                        slab[:],
                        w2_rs[s][:, ot].rearrange("p (ho oc) -> p ho oc", oc=P),
                    )
                    if cidx == 0:
                        # keep chunk-0 slabs off the W1 chase: start them
                        # only once mm1 of chunk 0 is well underway
                        tile.add_dep_helper(
                            dma.ins, acts[16 + 2 * ot].ins,
                            reason="pace first-chunk W2 slabs behind W1",
                        )
                    yp = yps.tile([P, CW], F32, tag="yp", name="yp")[:, :w]
                    for kt in range(HO):
                        nc.tensor.matmul(
                            yp[:],
                            slab[:, kt],
                            hgs[kt // HG][:, kt % HG],
                            start=(kt == 0),
                            stop=(kt == HO - 1),
                        )
                    o_sb = op.tile([P, CW], F32, tag="o", name="o_sb")[:, :w]
                    nc.vector.tensor_scalar_add(
                        o_sb[:], yp[:], b2_sb[s][:, ot:ot + 1])
                    nc.sync.dma_start(yT_r[:, ot, c0:c0 + w], o_sb[:])
                if is_last and s + 1 < NS:
                    emit_w1(s + 1, NW1 // 2, NW1)
    nc.finalize()
    return nc


def kernel(x, W1, b1, W2, b2, gate_w, gate_b):
    x = np.ascontiguousarray(x, dtype=np.float32)
    W1 = np.asarray(W1, dtype=np.float32)
    b1 = np.asarray(b1, dtype=np.float32)
    W2 = np.asarray(W2, dtype=np.float32)
    b2 = np.asarray(b2, dtype=np.float32)
    gate_w = np.ascontiguousarray(gate_w, dtype=np.float32)
    gate_b = np.asarray(gate_b, dtype=np.float32)

    B, D = x.shape
    E, _, H = W1.shape
    O = W2.shape[2]
    assert E == N_CORES and D % P == 0 and H % P == 0 and O % P == 0
    core_ids = list(range(N_CORES))

    # ---- Gate + routing on host (the sharding decision) ----
    logits = x @ gate_w + gate_b[None, :]
    lg = logits.astype(np.float64)
    lg -= lg.max(axis=1, keepdims=True)
    probs = np.exp(lg)
    probs /= probs.sum(axis=1, keepdims=True)
    order = np.argsort(-probs, axis=1, kind="stable")[:, :2]
    p_top = np.take_along_axis(probs, order, axis=1)
    w_top = (p_top / p_top.sum(axis=1, keepdims=True)).astype(np.float32)

    idx_e, wt_e = [], []
    for e in range(E):
        m0 = order[:, 0] == e
        m1 = order[:, 1] == e
        sel = m0 | m1
        idx = np.nonzero(sel)[0]
        wt = np.where(m0[sel], w_top[sel, 0], w_top[sel, 1]).astype(np.float32)
        idx_e.append(idx)
        wt_e.append(wt)
    counts = np.array([len(i) for i in idx_e])

    # ---- Snake experts (sorted by count) into N_GROUPS groups; each
    # group's EPG cores take 1/EPG slices of each of its experts ----
    order_e = list(np.argsort(-counts))
    groups = [[] for _ in range(N_GROUPS)]
    for r, e in enumerate(order_e):
        m = r % (2 * N_GROUPS)
        gi = m if m < N_GROUPS else 2 * N_GROUPS - 1 - m
        groups[gi].append(e)
    # slot i holds the i-th expert of each group; segment width is the
    # max 1/EPG slice across groups
    seg_w = [int(max(-(-counts[g[i]] // EPG) for g in groups))
             for i in range(EPG)]
    segs = tuple(tuple(_chunks_of(t)) for t in seg_w)

    key = ("expertN", D, H, O, segs)
    if key not in _build_cache:
        _build_cache[key] = _build_expertN(D, H, O, segs)
    nc = _build_cache[key]

    def w2_slabs(e):
        s = W2[e].reshape(H // P, P, O // P, P).transpose(2, 1, 0, 3)
        return np.ascontiguousarray(s.reshape(O, H)).astype(_bf16_np)

    TT = sum(seg_w)
    seg_off = np.concatenate([[0], np.cumsum(seg_w)]).astype(int)
    xT_bf = np.ascontiguousarray(x.T).astype(_bf16_np)  # [D, B]
    in_maps, slices = [], []
    for gi, grp in enumerate(groups):
        wmaps = {}
        for si, e in enumerate(grp):
            wmaps[f"w1_{si}"] = W1[e].astype(_bf16_np)
            wmaps[f"w2_{si}"] = w2_slabs(e)
            wmaps[f"b1_{si}"] = np.ascontiguousarray(b1[e].reshape(H // P, P).T)
            wmaps[f"b2_{si}"] = np.ascontiguousarray(b2[e].reshape(O // P, P).T)
        for q in range(EPG):
            xcm = np.zeros((D, TT), dtype=_bf16_np)
            core_slices = []
            for si, e in enumerate(grp):
                qe = -(-counts[e] // EPG)
                sl = idx_e[e][q * qe:(q + 1) * qe]
                wl = wt_e[e][q * qe:(q + 1) * qe]
                xcm[:, seg_off[si]:seg_off[si] + len(sl)] = xT_bf[:, sl]
                core_slices.append((sl, wl, int(seg_off[si])))
            in_maps.append(dict(wmaps, xc=xcm))
            slices.append(core_slices)
    res = run_bass_kernel_spmd(nc, in_maps, core_ids=core_ids)

    # ---- Host: gate-weight scale, transpose, scatter-add ----
    out = np.zeros((B, O), dtype=np.float32)
    for c, core_slices in enumerate(slices):
        yTo = res.results[c]["yT"]  # [O, TT] f32
        for sl, wl, off in core_slices:
            if len(sl):
                out[sl] += yTo[:, off:off + len(sl)].T * wl[:, None]
    return out
